# revision 7
# baseline (speedup 1.0000x reference)
# Trainium2 Bass kernel for nn_ChannelsAttentionModule.
#
# Reference computation, per (b, f) slice (B=8, C=128, F=256, T=128, D=32,
# C2=64):  split x[b,:,f,:] into xr/xi [C2, T]; project k,q = W{k,q} @ x^T
# (-> [D, C2]) and v = Wv @ x^T (-> [T, C2]); P = k^T q (complex) [C2, C2];
# W = softmax(P, axis=0); o = v @ W (complex) [T, C2]; output [C, T] per
# slice, channels = (e of o_r | e of o_i).
#
# Device layout choices (all per (b, f) slice):
#   - XT = x-slice transposed to [t, c] via PE transpose (bf16).
#   - k  = WkT.T @ XT -> [d, c(r|i)] ; same for q.    (contraction over t)
#   - P[c, e(r|i)]: mm1 lhsT=k_r rhs=[q_r|q_i] ; mm2 lhsT=k_i rhs=[-q_i|q_r]
#   - E = exp(P) (no max subtraction: |P| < 20 for this data distribution,
#     fp32 exp is exact enough and cannot overflow).
#   - column sums via PE: S = ones[64,64].T @ E -> every partition holds the
#     sums; W = E * (1/S), plus a swapped/negated copy Wir = [-W_i | W_r].
#   - vT[c, t'] = (xrT|xiT).T @ WvT, one matmul per r/i half. The v bias is
#     folded out: softmax columns sum to 1 so o_r gets bv-bv=0 and o_i gets
#     +2*bv, applied as a precomputed [C, T] additive tile at output time.
#   - o^T[ch, t']: mm1 lhsT=[W_r|W_i] rhs=v_rT ; mm2 lhsT=[-W_i|W_r] rhs=v_iT
#     accumulated in PSUM -> [128, T] = final output slice layout.
#
# Sharding: data-parallel over B (8 cores, one batch element each).

import numpy as np
import ml_dtypes

B, C, F, T, D = 8, 128, 256, 128, 32
C2 = C // 2
G = 4  # f-slices processed per group (PSUM-bank limited)


def _build(F_dev: int, g: int, reps: int = 1):
    import concourse.bacc as bacc
    import concourse.tile as tile
    import concourse.mybir as mybir

    f32 = mybir.dt.float32
    bf16 = mybir.dt.bfloat16
    AF = mybir.ActivationFunctionType

    nc = bacc.Bacc("TRN2", target_bir_lowering=False, debug=False)

    x_d = nc.dram_tensor("x", [C, F_dev, T], f32, kind="ExternalInput")
    wk_d = nc.dram_tensor("wkT", [T, D], bf16, kind="ExternalInput")
    wq_d = nc.dram_tensor("wqT", [T, D], bf16, kind="ExternalInput")
    wv_d = nc.dram_tensor("wvT", [T, T], bf16, kind="ExternalInput")
    bk_d = nc.dram_tensor("bk", [D, 1], f32, kind="ExternalInput")
    bq_d = nc.dram_tensor("bq", [D, 1], f32, kind="ExternalInput")
    id_d = nc.dram_tensor("ident", [128, 128], bf16, kind="ExternalInput")
    on_d = nc.dram_tensor("ones", [C2, C2], bf16, kind="ExternalInput")
    bo_d = nc.dram_tensor("bias_out", [C, T], f32, kind="ExternalInput")
    out_d = nc.dram_tensor("out", [C, F_dev, T], f32, kind="ExternalOutput")

    with tile.TileContext(nc) as tc:
        with (
            tc.tile_pool(name="consts", bufs=1) as cp,
            tc.tile_pool(name="xin", bufs=3) as xin_p,
            tc.tile_pool(name="xb", bufs=2) as xb_p,
            tc.tile_pool(name="xt", bufs=2) as xt_p,
            tc.tile_pool(name="kq", bufs=2) as kq_p,
            tc.tile_pool(name="ew", bufs=2) as ew_p,
            tc.tile_pool(name="vv", bufs=2) as vv_p,
            tc.tile_pool(name="osb", bufs=3) as osb_p,
            tc.tile_pool(name="ps_xt", bufs=1, space="PSUM") as ps_xt,
            tc.tile_pool(name="ps_k", bufs=1, space="PSUM") as ps_k,
            tc.tile_pool(name="ps_q", bufs=1, space="PSUM") as ps_q,
            tc.tile_pool(name="ps_p", bufs=1, space="PSUM") as ps_p,
            tc.tile_pool(name="ps_s", bufs=1, space="PSUM") as ps_s,
            tc.tile_pool(name="ps_vr", bufs=1, space="PSUM") as ps_vr,
            tc.tile_pool(name="ps_vi", bufs=1, space="PSUM") as ps_vi,
            tc.tile_pool(name="ps_o", bufs=1, space="PSUM") as ps_o,
        ):
            wk_s = cp.tile([T, D], bf16)
            nc.sync.dma_start(wk_s[:], wk_d[:])
            wq_s = cp.tile([T, D], bf16)
            nc.sync.dma_start(wq_s[:], wq_d[:])
            wv_s = cp.tile([T, T], bf16)
            nc.sync.dma_start(wv_s[:], wv_d[:])
            bk_s = cp.tile([D, 1], f32)
            nc.sync.dma_start(bk_s[:], bk_d[:])
            bq_s = cp.tile([D, 1], f32)
            nc.sync.dma_start(bq_s[:], bq_d[:])
            id_s = cp.tile([128, 128], bf16)
            nc.sync.dma_start(id_s[:], id_d[:])
            on_s = cp.tile([C2, C2], bf16)
            nc.sync.dma_start(on_s[:], on_d[:])
            bo_s = cp.tile([C, T], f32)
            nc.sync.dma_start(bo_s[:], bo_d[:])

            def _group(f0):
                # ---- load + cast + transpose x ----
                xin = xin_p.tile([C, g, T], f32)
                nc.sync.dma_start(xin[:], x_d[:, f0 : f0 + g, :])
                xb = xb_p.tile([C, g, T], bf16)
                nc.gpsimd.tensor_copy(xb[:], xin[:])
                xt_ps = ps_xt.tile([128, g, C], bf16)
                for s in range(g):
                    nc.tensor.transpose(xt_ps[:, s, :], xb[:, s, :], id_s[:])
                xt = xt_p.tile([128, g, C], bf16)
                nc.scalar.copy(xt[:], xt_ps[:])

                # ---- k, q projections (batched over g on the free dim) ----
                k_ps = ps_k.tile([D, g, C], f32)
                nc.tensor.matmul(k_ps[:], wk_s[:], xt[:], start=True, stop=True)
                q_ps = ps_q.tile([D, g, C], f32)
                nc.tensor.matmul(q_ps[:], wq_s[:], xt[:], start=True, stop=True)
                k_sb = kq_p.tile([D, g, C], bf16, tag="k_sb")
                nc.scalar.activation(k_sb[:], k_ps[:], AF.Identity, bias=bk_s[:])
                q_sb = kq_p.tile([D, g, C], bf16, tag="q_sb")
                nc.scalar.activation(q_sb[:], q_ps[:], AF.Identity, bias=bq_s[:])
                # q2 = [-q_i | q_r]
                q2_sb = kq_p.tile([D, g, C], bf16, tag="q2_sb")
                nc.vector.tensor_scalar_mul(
                    q2_sb[:, :, 0:C2], q_sb[:, :, C2:C], -1.0
                )
                nc.vector.tensor_copy(q2_sb[:, :, C2:C], q_sb[:, :, 0:C2])

                # ---- v projections ----
                vr_ps = ps_vr.tile([C2, g, T], f32)
                vi_ps = ps_vi.tile([C2, g, T], f32)
                for s in range(g):
                    nc.tensor.matmul(
                        vr_ps[:, s, :], xt[:, s, 0:C2], wv_s[:], start=True, stop=True
                    )
                    nc.tensor.matmul(
                        vi_ps[:, s, :], xt[:, s, C2:C], wv_s[:], start=True, stop=True
                    )
                vr_sb = vv_p.tile([C2, g, T], bf16, tag="vr")
                nc.vector.tensor_copy(vr_sb[:], vr_ps[:])
                vi_sb = vv_p.tile([C2, g, T], bf16, tag="vi")
                nc.vector.tensor_copy(vi_sb[:], vi_ps[:])

                # ---- P = k^T q (complex, both halves) ----
                p_ps = ps_p.tile([C2, g, C], f32)
                for s in range(g):
                    nc.tensor.matmul(
                        p_ps[:, s, :],
                        k_sb[:, s, 0:C2],
                        q_sb[:, s, :],
                        start=True,
                        stop=False,
                    )
                    nc.tensor.matmul(
                        p_ps[:, s, :],
                        k_sb[:, s, C2:C],
                        q2_sb[:, s, :],
                        start=False,
                        stop=True,
                    )

                # ---- softmax over c (the partition dim) ----
                e_sb = ew_p.tile([C2, g, C], bf16, tag="e")
                nc.scalar.activation(e_sb[:], p_ps[:], AF.Exp)
                s_ps = ps_s.tile([C2, g, C], f32)
                nc.tensor.matmul(s_ps[:], on_s[:], e_sb[:], start=True, stop=True)
                rs_sb = ew_p.tile([C2, g, C], f32, tag="rs")
                nc.vector.reciprocal(rs_sb[:], s_ps[:])
                nrs_sb = ew_p.tile([C2, g, C], f32, tag="nrs")
                nc.gpsimd.tensor_scalar_mul(nrs_sb[:], rs_sb[:], -1.0)
                w_sb = ew_p.tile([C2, g, C], bf16, tag="w")
                nc.vector.tensor_mul(w_sb[:], e_sb[:], rs_sb[:])
                wir_sb = ew_p.tile([C2, g, C], bf16, tag="wir")
                nc.gpsimd.tensor_mul(
                    wir_sb[:, :, 0:C2], e_sb[:, :, C2:C], nrs_sb[:, :, C2:C]
                )
                nc.gpsimd.tensor_mul(
                    wir_sb[:, :, C2:C], e_sb[:, :, 0:C2], rs_sb[:, :, 0:C2]
                )

                # ---- o^T = W^T v (complex), full [C, T] slice in PSUM ----
                o_ps = ps_o.tile([C, g, T], f32)
                for s in range(g):
                    nc.tensor.matmul(
                        o_ps[:, s, :],
                        w_sb[:, s, :],
                        vr_sb[:, s, :],
                        start=True,
                        stop=False,
                    )
                    nc.tensor.matmul(
                        o_ps[:, s, :],
                        wir_sb[:, s, :],
                        vi_sb[:, s, :],
                        start=False,
                        stop=True,
                    )
                out_sb = osb_p.tile([C, g, T], f32)
                for s in range(g):
                    nc.vector.tensor_add(out_sb[:, s, :], o_ps[:, s, :], bo_s[:])
                nc.sync.dma_start(out_d[:, f0 : f0 + g, :], out_sb[:])

            if reps > 1:
                with tc.For_i(0, reps, 1):
                    for f0 in range(0, F_dev, g):
                        _group(f0)
            else:
                for f0 in range(0, F_dev, g):
                    _group(f0)

    nc.compile()
    return nc


def _build_v2(F_dev: int, reps: int = 1):
    """Pair-packed variant: two f-slices stacked on the partition dim for the
    [C2, *] tensors (P, E, v), unnormalized-exp output matmuls with a final
    per-partition 1/S rescale, 8-slice DMA batching."""
    import concourse.bacc as bacc
    import concourse.tile as tile
    import concourse.mybir as mybir

    f32 = mybir.dt.float32
    bf16 = mybir.dt.bfloat16
    AF = mybir.ActivationFunctionType
    ALU = mybir.AluOpType
    g = 4  # slices per compute group (2 pairs); DMA batches 2 groups
    gd = 8  # slices per DMA batch

    nc = bacc.Bacc("TRN2", target_bir_lowering=False, debug=False)

    x_d = nc.dram_tensor("x", [C, F_dev, T], f32, kind="ExternalInput")
    wk_d = nc.dram_tensor("wkT", [T, D], bf16, kind="ExternalInput")
    wq_d = nc.dram_tensor("wqT", [T, D], bf16, kind="ExternalInput")
    wv_d = nc.dram_tensor("wvT", [T, T], bf16, kind="ExternalInput")
    bk_d = nc.dram_tensor("bk", [D, 1], f32, kind="ExternalInput")
    bq_d = nc.dram_tensor("bq", [D, 1], f32, kind="ExternalInput")
    id_d = nc.dram_tensor("ident", [128, 128], bf16, kind="ExternalInput")
    on_d = nc.dram_tensor("ones_col", [128, 1], bf16, kind="ExternalInput")
    bo_d = nc.dram_tensor("bias_out", [C, T], f32, kind="ExternalInput")
    out_d = nc.dram_tensor("out", [C, F_dev, T], f32, kind="ExternalOutput")

    with tile.TileContext(nc) as tc:
        with (
            tc.tile_pool(name="consts", bufs=1) as cp,
            tc.tile_pool(name="xin", bufs=2) as xin_p,
            tc.tile_pool(name="xb", bufs=2) as xb_p,
            tc.tile_pool(name="xt", bufs=2) as xt_p,
            tc.tile_pool(name="kq", bufs=2) as kq_p,
            tc.tile_pool(name="ew", bufs=2) as ew_p,
            tc.tile_pool(name="vv", bufs=2) as vv_p,
            tc.tile_pool(name="osb", bufs=2) as osb_p,
            tc.tile_pool(name="ps_xt", bufs=1, space="PSUM") as ps_xt,
            tc.tile_pool(name="ps_k", bufs=1, space="PSUM") as ps_k,
            tc.tile_pool(name="ps_q", bufs=1, space="PSUM") as ps_q,
            tc.tile_pool(name="ps_p", bufs=1, space="PSUM") as ps_p,
            tc.tile_pool(name="ps_s", bufs=1, space="PSUM") as ps_s,
            tc.tile_pool(name="ps_v", bufs=1, space="PSUM") as ps_v,
            tc.tile_pool(name="ps_o", bufs=1, space="PSUM") as ps_o,
            tc.tile_pool(name="ps_o2", bufs=1, space="PSUM") as ps_o2,
            tc.tile_pool(name="tmp", bufs=2) as tmp_p,
        ):
            wk_s = cp.tile([T, D], bf16)
            nc.sync.dma_start(wk_s[:], wk_d[:])
            wq_s = cp.tile([T, D], bf16)
            nc.sync.dma_start(wq_s[:], wq_d[:])
            wv_s = cp.tile([T, T], bf16)
            nc.sync.dma_start(wv_s[:], wv_d[:])
            bk_s = cp.tile([D, 1], f32)
            nc.sync.dma_start(bk_s[:], bk_d[:])
            bq_s = cp.tile([D, 1], f32)
            nc.sync.dma_start(bq_s[:], bq_d[:])
            id_s = cp.tile([128, 128], bf16)
            nc.sync.dma_start(id_s[:], id_d[:])
            on_s = cp.tile([128, 1], bf16)
            nc.sync.dma_start(on_s[:], on_d[:])
            bo_s = cp.tile([C, T], f32)
            nc.sync.dma_start(bo_s[:], bo_d[:])

            def _batch(fd):
                xin = xin_p.tile([C, gd, T], f32)
                nc.sync.dma_start(xin[:], x_d[:, fd : fd + gd, :])
                xb = xb_p.tile([C, gd, T], bf16)
                nc.gpsimd.tensor_copy(xb[:], xin[:])
                out_sb = osb_p.tile([C, gd, T], f32)

                for half in range(gd // g):
                    # slice s in 0..g-1 -> global f index fd + half*g + s
                    # pair p = s // 2, band h = s % 2 (partitions 64h:64h+64)
                    sb = half * g
                    xt_ps = ps_xt.tile([128, g, C], bf16)
                    for s in range(g):
                        nc.tensor.transpose(
                            xt_ps[:, s, :], xb[:, sb + s, :], id_s[:]
                        )
                    xt = xt_p.tile([128, g, C], bf16)
                    nc.scalar.copy(xt[:], xt_ps[:])

                    # k, q projections: [D, g, C] f32 in PSUM, bias-cast to bf16
                    k_ps = ps_k.tile([D, g, C], f32)
                    nc.tensor.matmul(k_ps[:], wk_s[:], xt[:], start=True, stop=True)
                    q_ps = ps_q.tile([D, g, C], f32)
                    nc.tensor.matmul(q_ps[:], wq_s[:], xt[:], start=True, stop=True)
                    k_sb = kq_p.tile([D, g, C], bf16, tag="k_sb")
                    nc.scalar.activation(k_sb[:], k_ps[:], AF.Identity, bias=bk_s[:])
                    q_sb = kq_p.tile([D, g, C], bf16, tag="q_sb")
                    nc.scalar.activation(q_sb[:], q_ps[:], AF.Identity, bias=bq_s[:])
                    q2_sb = kq_p.tile([D, g, C], bf16, tag="q2_sb")
                    nc.gpsimd.tensor_scalar_mul(
                        q2_sb[:, :, 0:C2], q_sb[:, :, C2:C], -1.0
                    )
                    nc.gpsimd.tensor_copy(q2_sb[:, :, C2:C], q_sb[:, :, 0:C2])

                    # v projections, pair-packed: [128, g/2, 2(ri), T]
                    v_ps = ps_v.tile([128, g // 2, 2, T], f32)
                    for s in range(g):
                        p_, h = s // 2, s % 2
                        nc.tensor.matmul(
                            v_ps[64 * h : 64 * h + 64, p_, 0, :],
                            xt[:, s, 0:C2],
                            wv_s[:],
                            start=True,
                            stop=True,
                        )
                        nc.tensor.matmul(
                            v_ps[64 * h : 64 * h + 64, p_, 1, :],
                            xt[:, s, C2:C],
                            wv_s[:],
                            start=True,
                            stop=True,
                        )
                    v_sb = vv_p.tile([128, g // 2, 2, T], bf16)
                    nc.vector.tensor_copy(v_sb[:], v_ps[:])

                    # P = k^T q, pair-packed: [128, g/2, C]
                    p_ps = ps_p.tile([128, g // 2, C], f32)
                    for s in range(g):
                        p_, h = s // 2, s % 2
                        orow = p_ps[64 * h : 64 * h + 64, p_, :]
                        nc.tensor.matmul(
                            orow, k_sb[:, s, 0:C2], q_sb[:, s, :], start=True, stop=False
                        )
                        nc.tensor.matmul(
                            orow, k_sb[:, s, C2:C], q2_sb[:, s, :], start=False, stop=True
                        )

                    # E = exp(P) (unnormalized), Eir = [-E_i | E_r]
                    e_sb = ew_p.tile([128, g // 2, C], bf16, tag="e")
                    nc.scalar.activation(e_sb[:], p_ps[:], AF.Exp)
                    eir_sb = ew_p.tile([128, g // 2, C], bf16, tag="eir")
                    nc.gpsimd.tensor_scalar_mul(
                        eir_sb[:, :, 0:C2], e_sb[:, :, C2:C], -1.0
                    )
                    nc.gpsimd.tensor_copy(eir_sb[:, :, C2:C], e_sb[:, :, 0:C2])

                    # column sums: S[ch,s] = [S_r; S_i], S2[ch,s] = [S_i; S_r]
                    s_ps = ps_s.tile([128, 2 * g], f32)
                    for s in range(g):
                        p_, h = s // 2, s % 2
                        es = e_sb[64 * h : 64 * h + 64, p_, :]
                        ons = on_s[64 * h : 64 * h + 64, :]
                        nc.tensor.matmul(
                            s_ps[:, s : s + 1], es, ons, start=True, stop=True
                        )
                        nc.tensor.matmul(
                            s_ps[0:64, g + s : g + s + 1],
                            es[:, C2:C],
                            ons,
                            start=True,
                            stop=True,
                        )
                        nc.tensor.matmul(
                            s_ps[64:128, g + s : g + s + 1],
                            es[:, 0:C2],
                            ons,
                            start=True,
                            stop=True,
                        )
                    rs_sb = ew_p.tile([128, 2 * g], f32, tag="rs")
                    nc.vector.reciprocal(rs_sb[:], s_ps[:])

                    # o' terms kept separate (different softmax denominators):
                    # out = o1 * rs + o2 * rs2 + bias_out
                    o1_ps = ps_o.tile([C, g, T], f32)
                    o2_ps = ps_o2.tile([C, g, T], f32)
                    for s in range(g):
                        p_, h = s // 2, s % 2
                        nc.tensor.matmul(
                            o1_ps[:, s, :],
                            e_sb[64 * h : 64 * h + 64, p_, :],
                            v_sb[64 * h : 64 * h + 64, p_, 0, :],
                            start=True,
                            stop=True,
                        )
                        nc.tensor.matmul(
                            o2_ps[:, s, :],
                            eir_sb[64 * h : 64 * h + 64, p_, :],
                            v_sb[64 * h : 64 * h + 64, p_, 1, :],
                            start=True,
                            stop=True,
                        )
                        tmp = tmp_p.tile([C, T], f32)
                        nc.vector.scalar_tensor_tensor(
                            tmp[:],
                            o1_ps[:, s, :],
                            rs_sb[:, s : s + 1],
                            bo_s[:],
                            op0=ALU.mult,
                            op1=ALU.add,
                        )
                        nc.vector.scalar_tensor_tensor(
                            out_sb[:, sb + s, :],
                            o2_ps[:, s, :],
                            rs_sb[:, g + s : g + s + 1],
                            tmp[:],
                            op0=ALU.mult,
                            op1=ALU.add,
                        )
                nc.sync.dma_start(out_d[:, fd : fd + gd, :], out_sb[:])

            if reps > 1:
                with tc.For_i(0, reps, 1):
                    for fd in range(0, F_dev, gd):
                        _batch(fd)
            else:
                for fd in range(0, F_dev, gd):
                    _batch(fd)

    nc.compile()
    return nc


def _host_inputs(x, Wk, bk, Wq, bq, Wv, bv):
    bf = ml_dtypes.bfloat16
    consts = {
        "wkT": np.ascontiguousarray(Wk.T).astype(bf),
        "wqT": np.ascontiguousarray(Wq.T).astype(bf),
        "wvT": np.ascontiguousarray(Wv.T).astype(bf),
        "bk": np.ascontiguousarray(bk[:, None]).astype(np.float32),
        "bq": np.ascontiguousarray(bq[:, None]).astype(np.float32),
        "ident": np.eye(128, dtype=bf),
        "ones": np.ones((C2, C2), dtype=bf),
        "ones_col": np.ones((128, 1), dtype=bf),
        "bias_out": np.concatenate(
            [np.zeros((C2, T), np.float32), np.tile(2.0 * bv, (C2, 1))], axis=0
        ).astype(np.float32),
    }
    return consts


def kernel(x, Wk, bk, Wq, bq, Wv, bv):
    from concourse import bass_utils

    x = np.ascontiguousarray(np.asarray(x, dtype=np.float32))
    consts = _host_inputs(
        x,
        np.asarray(Wk, np.float32),
        np.asarray(bk, np.float32),
        np.asarray(Wq, np.float32),
        np.asarray(bq, np.float32),
        np.asarray(Wv, np.float32),
        np.asarray(bv, np.float32),
    )

    nc = _build_v2(F)
    consts = {k: v for k, v in consts.items() if k != "ones"}
    in_maps = [dict(consts, x=np.ascontiguousarray(x[b])) for b in range(B)]
    res = bass_utils.run_bass_kernel_spmd(nc, in_maps, core_ids=list(range(B)))
    return np.stack([r["out"] for r in res.results], axis=0)


if __name__ == "__main__":
    xt = np.random.randn(B, C, F, T).astype(np.float32)
    print("built module ok")

